# revision 1
# baseline (speedup 1.0000x reference)
"""Trainium2 Bass kernel for nn_CrossLayer (dense transformer layer).

Sharding: sequence-parallel over 8 cores (2 samples x 4 token-chunks of 512).
Each core computes its 512 token rows through CA -> SA -> FFN. K/V for all 16
heads are computed from each core's own rows and AllGather'd (bf16) across the
4 cores of its sample, once per attention block.

On-chip layout: activations feature-major [dim(128p x 8c), tok] so every
matmul contracts over partitions. RMSNorm partition-sums via ones-matmuls on
PE; RoPE rotate-half via a constant +-1 block matrix on PE; softmax
denominators via an appended ones column on V; exp without max subtraction
(scores are O(1): q/k are rms-normalized and /sqrt(d)).
"""

import math
import sys
import types

import numpy as np
import ml_dtypes

B, N, DIM, HID, H, D = 2, 2048, 1024, 4096, 16, 64
TOK = 512  # tokens per core
NCORES = 8
EPS = 1e-6
THETA = 10000.0
P = 128
KO = DIM // P  # 8 contraction chunks
HH = H // 2  # 8 head pairs
HC = HID // P  # 32 hidden chunks
TC = TOK // P  # 4 token chunks per core
NR = 4  # ranks per replica group
VW = D + 1  # v columns + ones column

BF = ml_dtypes.bfloat16

_cache = {}


def _lhsT_layout(W):
    """[K, M] -> [M//128, 128(K%128), K//128, 128(M%128)]: SBUF slices are
    matmul lhsT tiles [128, 128]."""
    K, M = W.shape
    return (
        W.reshape(K // P, P, M // P, P).transpose(2, 1, 0, 3).astype(BF).copy()
    )


def _rhs_layout(W):
    """[K, M] -> [128, K//128, M] rhs-style."""
    K, M = W.shape
    return W.reshape(K // P, P, M).transpose(1, 0, 2).astype(BF).copy()


def _featmajor(x):
    """[tok, dim] -> [128, dim//128, tok] float32."""
    return x.T.reshape(DIM // P, P, x.shape[0]).transpose(1, 0, 2).copy()


def _rope_tables(pos):
    """pos [TOK] int32 -> cos/sin [128, TOK] (2 heads stacked) bf16."""
    invf = 1.0 / (THETA ** (np.arange(0, D, 2, dtype=np.float64) / D))  # [32]
    ang = pos.astype(np.float64)[None, :] * invf[:, None]  # [32, TOK]
    c = np.cos(ang)
    s = np.sin(ang)
    c64 = np.concatenate([c, c], axis=0)  # [64, TOK]
    s64 = np.concatenate([s, s], axis=0)
    c128 = np.concatenate([c64, c64], axis=0).astype(BF)  # [128, TOK]
    s128 = np.concatenate([s64, s64], axis=0).astype(BF)
    return c128.copy(), s128.copy()


def _install_ntff_hook():
    try:
        from trn_agent_boot.trn_boot import _ntff_profile_via_ctypes
    except ImportError:
        return
    if "antenv.axon_hooks" in sys.modules:
        return
    try:
        hook = _ntff_profile_via_ctypes("/opt/axon/libaxon_pjrt.so")
    except OSError:
        return
    mod = types.ModuleType("antenv.axon_hooks")
    mod.get_axon_ntff_profile_hook = lambda: hook
    mod.set_axon_ntff_profile_hook = lambda h: None
    sys.modules["antenv.axon_hooks"] = mod
    import antenv

    antenv.axon_hooks = mod


def _split_multiwait(nc):
    """This walrus only supports one sync-wait on CTRL-encoded instructions
    (Drain/NoOp); hoist excess waits onto single-wait NoOps placed before."""
    from concourse import mybir

    n_split = 0
    for f in nc.m.functions:
        for bb in f.blocks:
            new = []
            changed = False
            for ins in bb.instructions:
                si = ins.sync_info
                if (
                    si is not None
                    and si.on_wait is not None
                    and len(si.on_wait) > 1
                ):
                    waits = list(si.on_wait)
                    keep, rest = waits[:1], waits[1:]
                    for k, w in enumerate(rest):
                        new.append(
                            mybir.InstNoOp(
                                name=f"{ins.name}-wsplit{k}",
                                engine=ins.engine,
                                sync_info=mybir.SyncInfo(
                                    on_wait=[w], on_update=[]
                                ),
                                bass_nofuse=True,
                            )
                        )
                    si.on_wait = keep
                    n_split += 1
                    changed = True
                new.append(ins)
            if changed:
                bb.instructions = new
    return n_split


def _build_bass():
    from contextlib import ExitStack

    import concourse.bass as bass
    import concourse.tile as tile
    from concourse import mybir

    f32 = mybir.dt.float32
    bf16 = mybir.dt.bfloat16
    AF = mybir.ActivationFunctionType

    nc = bass.Bass(num_devices=NCORES)

    def inp(name, shape, dt=bf16):
        return nc.dram_tensor(name, shape, dt, kind="ExternalInput")

    tgtT = inp("tgtT", [P, KO, TOK], f32)
    srcTb = inp("srcTb", [P, KO, TOK])
    cosq = inp("cosq", [P, TOK])
    sinq = inp("sinq", [P, TOK])
    coskca = inp("coskca", [P, TOK])
    sinkca = inp("sinkca", [P, TOK])
    caWq = inp("caWq", [HH, P, KO, P])
    caWk = inp("caWk", [HH, P, KO, P])
    caWv = inp("caWv", [P, KO, DIM])
    caWo = inp("caWo", [KO, P, KO, P])
    saWq = inp("saWq", [HH, P, KO, P])
    saWk = inp("saWk", [HH, P, KO, P])
    saWv = inp("saWv", [P, KO, DIM])
    saWo = inp("saWo", [KO, P, KO, P])
    W1i = inp("W1", [HC, P, KO, P])
    W3i = inp("W3", [HC, P, KO, P])
    W2i = inp("W2", [KO, P, HC, P])
    blk2 = inp("blk2", [P, 2])  # per-head ssq lhsT (block ones)
    mq_ca = inp("mq_ca", [2, P])  # rsqrt bcast lhsT with qn folded
    mk_ca = inp("mk_ca", [2, P])
    mq_sa = inp("mq_sa", [2, P])
    mk_sa = inp("mk_sa", [2, P])
    rotm = inp("rotm", [P, P])  # rotate-half (2-head block diag) lhsT
    ones_c = inp("ones_c", [P, 1])  # y-norm ssq lhsT
    ones_r128 = inp("ones_r128", [1, P])  # y-norm bcast lhsT

    outT = nc.dram_tensor("outT", [P, KO, TOK], f32, kind="ExternalOutput")

    groups = [[0, 1, 2, 3], [4, 5, 6, 7]]
    KWORDS = P * HH * TOK  # k bf16 words per rank
    VWORDS = P * TC * H * VW  # v bf16 words per rank

    with tile.TileContext(nc) as tc:
        ctx = ExitStack()
        with ctx:
            sing = ctx.enter_context(tc.tile_pool(name="sing", bufs=1))
            wpool = ctx.enter_context(tc.tile_pool(name="wpool", bufs=2))
            w2pool = ctx.enter_context(tc.tile_pool(name="w2pool", bufs=2))
            work = ctx.enter_context(tc.tile_pool(name="work", bufs=3))
            probp = ctx.enter_context(tc.tile_pool(name="probp", bufs=2))
            stat = ctx.enter_context(tc.tile_pool(name="stat", bufs=2))
            kvpool = ctx.enter_context(tc.tile_pool(name="kvpool", bufs=1))
            dram = ctx.enter_context(
                tc.tile_pool(name="dram", bufs=1, space="DRAM")
            )
            pp = ctx.enter_context(tc.tile_pool(name="pp", bufs=2, space="PSUM"))
            ps_s = ctx.enter_context(
                tc.tile_pool(name="ps_s", bufs=2, space="PSUM")
            )
            ps_x = ctx.enter_context(
                tc.tile_pool(name="ps_x", bufs=1, space="PSUM")
            )

            # ---- resident tiles
            resid = sing.tile([P, KO, TOK], f32)
            nc.sync.dma_start(resid[:], tgtT[:])
            srcT_sb = kvpool.tile([P, KO, TOK], bf16, tag="xT", name="srcT_sb")
            nc.sync.dma_start(srcT_sb[:], srcTb[:])
            cosq_sb = sing.tile([P, TOK], bf16)
            nc.sync.dma_start(cosq_sb[:], cosq[:])
            sinq_sb = sing.tile([P, TOK], bf16)
            nc.sync.dma_start(sinq_sb[:], sinq[:])
            coskca_sb = sing.tile([P, TOK], bf16)
            nc.sync.dma_start(coskca_sb[:], coskca[:])
            sinkca_sb = sing.tile([P, TOK], bf16)
            nc.sync.dma_start(sinkca_sb[:], sinkca[:])
            blk2_sb = sing.tile([P, 2], bf16)
            nc.sync.dma_start(blk2_sb[:], blk2[:])
            masks_sb = {}
            for name, t in (
                ("mq_ca", mq_ca),
                ("mk_ca", mk_ca),
                ("mq_sa", mq_sa),
                ("mk_sa", mk_sa),
            ):
                m = sing.tile([2, P], bf16, name=name)
                nc.sync.dma_start(m[:], t[:])
                masks_sb[name] = m
            rotm_sb = sing.tile([P, P], bf16)
            nc.sync.dma_start(rotm_sb[:], rotm[:])
            ones_c_sb = sing.tile([P, 1], bf16)
            nc.sync.dma_start(ones_c_sb[:], ones_c[:])
            ones_r128_sb = sing.tile([1, P], bf16)
            nc.sync.dma_start(ones_r128_sb[:], ones_r128[:])
            eps_sb = sing.tile([2, 1], mybir.dt.float32)
            nc.vector.memset(eps_sb[:], float(EPS))

            def norm_rope_one(psum_q, mask_sb, cos_sb, sin_sb, dst):
                """psum_q [128(2 heads), TOK] f32 -> dst bf16: rms-normed,
                qn-scaled, roped."""
                raw = stat.tile([P, TOK], f32, tag="raw", name="raw")
                nc.vector.tensor_copy(raw[:], psum_q[:])
                sq = work.tile([P, TOK], bf16, tag="ysq", name="sq")
                nc.vector.tensor_mul(sq[:], raw[:], raw[:])
                ssq = pp.tile([2, TOK], f32, tag="pp", name="ssq")
                nc.tensor.matmul(ssq[:], blk2_sb[:], sq[:], start=True, stop=True)
                # rsqrt(mean+eps) = exp(-0.5*ln(mean+eps)); Ln/Exp share one
                # ACT table set (natural_log_exp) with the attention exps
                lnt = stat.tile([2, TOK], f32, tag="lnt", name="lnt")
                nc.scalar.activation(
                    lnt[:], ssq[:], AF.Ln, bias=eps_sb[:], scale=1.0 / D
                )
                rs = stat.tile([2, TOK], bf16, tag="rs", name="rs")
                nc.scalar.activation(rs[:], lnt[:], AF.Exp, scale=-0.5)
                bc = pp.tile([P, TOK], f32, tag="pp", name="bc")
                nc.tensor.matmul(bc[:], mask_sb[:], rs[:], start=True, stop=True)
                v1 = stat.tile([P, TOK], bf16, tag="v1", name="v1")
                nc.vector.tensor_mul(v1[:], raw[:], bc[:])
                rot_ps = pp.tile([P, TOK], f32, tag="pp", name="rot_ps")
                nc.tensor.matmul(
                    rot_ps[:], rotm_sb[:], v1[:], start=True, stop=True
                )
                rot = stat.tile([P, TOK], bf16, tag="rot", name="rot")
                nc.scalar.copy(rot[:], rot_ps[:])
                t1 = stat.tile([P, TOK], bf16, tag="t1", name="t1")
                nc.vector.tensor_mul(t1[:], v1[:], cos_sb[:])
                nc.vector.tensor_mul(dst, rot[:], sin_sb[:])
                nc.vector.tensor_add(dst, t1[:], dst)

            def rmsnorm_feat(src_f32, dst_bf16):
                """Feature-major RMSNorm: dst = src * rsqrt(mean(src^2))."""
                ssq = pp.tile([1, TOK], f32, tag="pp", name="yssq")
                for c in range(KO):
                    sq = work.tile([P, TOK], bf16, tag="ysq", name="ynsq")
                    nc.vector.tensor_mul(sq[:], src_f32[:, c], src_f32[:, c])
                    nc.tensor.matmul(
                        ssq[:],
                        ones_c_sb[:],
                        sq[:],
                        start=(c == 0),
                        stop=(c == KO - 1),
                    )
                lnt = stat.tile([1, TOK], f32, tag="lnt", name="ylnt")
                nc.scalar.activation(
                    lnt[:], ssq[:], AF.Ln, bias=eps_sb[:1], scale=1.0 / DIM
                )
                rs = stat.tile([1, TOK], bf16, tag="rs", name="yrs")
                nc.scalar.activation(rs[:], lnt[:], AF.Exp, scale=-0.5)
                bc = pp.tile([P, TOK], f32, tag="pp", name="ybc")
                nc.tensor.matmul(
                    bc[:], ones_r128_sb[:], rs[:], start=True, stop=True
                )
                for c in range(KO):
                    nc.vector.tensor_mul(dst_bf16[:, c], src_f32[:, c], bc[:])

            def attention_block(y_sb, kvsrc_sb, Wq_t, Wk_t, Wv_t, Wo_t,
                                mq, mk, cosk, sink):
                """One attention block; y_sb bf16 [P,KO,TOK] is the q-side
                input, kvsrc_sb the kv-side input. Adds Wo output into resid."""
                # --- k projection + norm/rope from my rows
                k_mine = kvpool.tile([P, HH, TOK], bf16, tag="kq", name="k_mine")
                for g in range(2):  # stream Wk in halves
                    wk = wpool.tile([P, 4, KO, P], bf16, tag="w1m", name="wk")
                    nc.sync.dma_start(
                        wk[:],
                        Wk_t[g * 4 : (g + 1) * 4].rearrange(
                            "g p ko m -> p g ko m"
                        ),
                    )
                    for j in range(4):
                        hh = g * 4 + j
                        pk = pp.tile([P, TOK], f32, tag="pp", name="pk")
                        for c in range(KO):
                            nc.tensor.matmul(
                                pk[:],
                                wk[:, j, c],
                                kvsrc_sb[:, c],
                                start=(c == 0),
                                stop=(c == KO - 1),
                            )
                        norm_rope_one(pk, mk, cosk, sink, k_mine[:, hh])

                # --- v projection (token-major, with ones column)
                v_mine = kvpool.tile(
                    [P, TC, H, VW], bf16, tag="vm", name="v_mine"
                )
                nc.vector.memset(v_mine[:, :, :, D : D + 1], 1.0)
                for nh in range(2):
                    wv = wpool.tile([P, KO, TOK], bf16, tag="w1m", name="wv")
                    nc.sync.dma_start(
                        wv[:], Wv_t[:, :, nh * TOK : (nh + 1) * TOK]
                    )
                    for t4 in range(TC):
                        pv = pp.tile([P, TOK], f32, tag="pp", name="pv")
                        for c in range(KO):
                            nc.tensor.matmul(
                                pv[:],
                                kvsrc_sb[:, c, t4 * P : (t4 + 1) * P],
                                wv[:, c],
                                start=(c == 0),
                                stop=(c == KO - 1),
                            )
                        nc.vector.tensor_copy(
                            v_mine[:, t4, nh * 8 : (nh + 1) * 8, 0:D],
                            pv[:].rearrange("p (h d) -> p h d", d=D),
                        )

                # --- allgather k/v across my sample's 4 cores
                kv_in = dram.tile([KWORDS + VWORDS], bf16, tag="kv_in")
                nc.sync.dma_start(
                    kv_in[:KWORDS].rearrange(
                        "(p h t) -> p h t", p=P, h=HH, t=TOK
                    ),
                    k_mine[:],
                )
                nc.sync.dma_start(
                    kv_in[KWORDS:].rearrange(
                        "(p a b c) -> p a b c", p=P, a=TC, b=H, c=VW
                    ),
                    v_mine[:],
                )
                kv_out = dram.tile([NR, KWORDS + VWORDS], bf16, tag="kv_out")
                nc.gpsimd.collective_compute(
                    "AllGather",
                    mybir.AluOpType.bypass,
                    replica_groups=groups,
                    ins=[kv_in.opt()],
                    outs=[kv_out.opt()],
                )
                k_full = kvpool.tile(
                    [P, HH, NR, TOK], bf16, tag="k_full", name="k_full"
                )
                v_full = kvpool.tile(
                    [P, NR, TC, H, VW], bf16, tag="v_full", name="v_full"
                )
                for r in range(NR):
                    nc.sync.dma_start(
                        k_full[:, :, r],
                        kv_out[r, :KWORDS].rearrange(
                            "(p h t) -> p h t", p=P, h=HH, t=TOK
                        ),
                    )
                    nc.sync.dma_start(
                        v_full[:, r],
                        kv_out[r, KWORDS:].rearrange(
                            "(p a b c) -> p a b c", p=P, a=TC, b=H, c=VW
                        ),
                    )

                # --- q projection + norm + rope (overlaps the collective)
                q_sb = kvpool.tile([P, HH, TOK], bf16, tag="kq", name="q_sb")
                for g in range(2):
                    wq = wpool.tile([P, 4, KO, P], bf16, tag="w1m", name="wq")
                    nc.sync.dma_start(
                        wq[:],
                        Wq_t[g * 4 : (g + 1) * 4].rearrange(
                            "g p ko m -> p g ko m"
                        ),
                    )
                    for j in range(4):
                        hh = g * 4 + j
                        pq = pp.tile([P, TOK], f32, tag="pp", name="pq")
                        for c in range(KO):
                            nc.tensor.matmul(
                                pq[:],
                                wq[:, j, c],
                                y_sb[:, c],
                                start=(c == 0),
                                stop=(c == KO - 1),
                            )
                        norm_rope_one(pq, mq, cosq_sb, sinq_sb, q_sb[:, hh])

                # --- attention: 2 heads share one exp; denominators ride in
                # row 64 of the px accumulators (ones column of v)
                xT = kvpool.tile([P, HH, TOK], bf16, tag="xT", name="xT")
                dens = kvpool.tile([D + 1, H, TOK], bf16, tag="dens", name="dens")
                for hh in range(HH):
                    px = [
                        ps_x.tile([VW, TOK], f32, tag=f"px{i}", name=f"px{i}")
                        for i in range(2)
                    ]
                    for kc in range(H):  # 16 k-chunks of 128 tokens
                        r, tcl = kc // TC, kc % TC
                        ps = ps_s.tile([P, 2 * TOK], f32, tag="ps", name="ps")
                        for i in range(2):
                            off = i * D
                            nc.tensor.matmul(
                                ps[:, i * TOK : (i + 1) * TOK],
                                k_full[
                                    off : off + D,
                                    hh,
                                    r,
                                    tcl * P : (tcl + 1) * P,
                                ],
                                q_sb[off : off + D, hh],
                                start=True,
                                stop=True,
                            )
                        prob = probp.tile(
                            [P, 2 * TOK], bf16, tag="prob", name="prob"
                        )
                        nc.scalar.activation(
                            prob[:], ps[:], AF.Exp, scale=1.0 / math.sqrt(D)
                        )
                        for i in range(2):
                            h = hh * 2 + i
                            nc.tensor.matmul(
                                px[i][:],
                                v_full[:, r, tcl, h],
                                prob[:, i * TOK : (i + 1) * TOK],
                                start=(kc == 0),
                                stop=(kc == H - 1),
                            )
                    for i in range(2):
                        h = hh * 2 + i
                        # denom row lives on partition 64; keep it there
                        nc.vector.tensor_copy(
                            dens[D : D + 1, h], px[i][D : D + 1]
                        )
                        # 64-channel copy may retarget the other half-window
                        nc.vector.tensor_copy(
                            xT[i * D : (i + 1) * D, hh], px[i][0:D]
                        )

                # --- softmax denominators: one reciprocal, broadcast via DRAM
                dflat = dens[D : D + 1].rearrange("o h t -> o (h t)")
                nc.scalar.activation(dflat, dflat, AF.Ln)
                nc.scalar.activation(dflat, dflat, AF.Exp, scale=-1.0)
                db = dram.tile([H * TOK], bf16, tag="db")
                nc.sync.dma_start(
                    db[:].rearrange("(o h t) -> o h t", o=1, h=H),
                    dens[D : D + 1],
                )
                rec_bc = kvpool.tile(
                    [P, HH, TOK], bf16, tag="vm", name="rec_bc"
                )
                for i in range(2):
                    src = bass.AP(
                        tensor=db.tensor,
                        offset=db.offset + i * TOK,
                        ap=[[0, D], [2 * TOK, HH], [1, TOK]],
                    )
                    nc.sync.dma_start(rec_bc[i * D : (i + 1) * D], src)
                for hh in range(HH):
                    nc.vector.tensor_mul(
                        xT[:, hh], xT[:, hh], rec_bc[:, hh]
                    )

                # --- Wo projection, accumulate into resid
                for g in range(2):
                    wo = wpool.tile([P, 4, KO, P], bf16, tag="w1m", name="wo")
                    nc.sync.dma_start(
                        wo[:],
                        Wo_t[g * 4 : (g + 1) * 4].rearrange(
                            "g p ko m -> p g ko m"
                        ),
                    )
                    for j in range(4):
                        oc = g * 4 + j
                        po = pp.tile([P, TOK], f32, tag="pp", name="po")
                        for c in range(KO):
                            nc.tensor.matmul(
                                po[:],
                                wo[:, j, c],
                                xT[:, c],
                                start=(c == 0),
                                stop=(c == KO - 1),
                            )
                        nc.vector.tensor_add(resid[:, oc], resid[:, oc], po[:])

            # ================= cross-attention =================
            yT = sing.tile([P, KO, TOK], bf16, name="yT")
            rmsnorm_feat(resid, yT)
            attention_block(
                yT, srcT_sb, caWq, caWk, caWv, caWo,
                masks_sb["mq_ca"], masks_sb["mk_ca"], coskca_sb, sinkca_sb,
            )

            # ================= self-attention =================
            rmsnorm_feat(resid, yT)
            attention_block(
                yT, yT, saWq, saWk, saWv, saWo,
                masks_sb["mq_sa"], masks_sb["mk_sa"], cosq_sb, sinq_sb,
            )

            # ================= FFN =================
            rmsnorm_feat(resid, yT)
            hT = kvpool.tile([P, HC, TOK], bf16, tag="k_full", name="hT")
            for g in range(8):  # stream W1/W3 in eighths
                w1 = wpool.tile([P, 4, KO, P], bf16, tag="w1m", name="w1")
                nc.sync.dma_start(
                    w1[:],
                    W1i[g * 4 : (g + 1) * 4].rearrange("g p ko m -> p g ko m"),
                )
                w3 = wpool.tile([P, 4, KO, P], bf16, tag="w1m", name="w3")
                nc.sync.dma_start(
                    w3[:],
                    W3i[g * 4 : (g + 1) * 4].rearrange("g p ko m -> p g ko m"),
                )
                for j in range(4):
                    hc = g * 4 + j
                    p1 = pp.tile([P, TOK], f32, tag="pp", name="p1")
                    for c in range(KO):
                        nc.tensor.matmul(
                            p1[:], w1[:, j, c], yT[:, c],
                            start=(c == 0), stop=(c == KO - 1),
                        )
                    p3 = pp.tile([P, TOK], f32, tag="pp", name="p3")
                    for c in range(KO):
                        nc.tensor.matmul(
                            p3[:], w3[:, j, c], yT[:, c],
                            start=(c == 0), stop=(c == KO - 1),
                        )
                    s1 = stat.tile([P, TOK], f32, tag="raw", name="s1")
                    nc.scalar.activation(s1[:], p1[:], AF.Silu)
                    nc.vector.tensor_mul(hT[:, hc], s1[:], p3[:])
            for oc in range(KO):
                w2 = w2pool.tile([P, HC, P], bf16, tag="w2", name="w2")
                nc.sync.dma_start(w2[:], W2i[oc])
                po = pp.tile([P, TOK], f32, tag="pp", name="po2")
                for hc in range(HC):
                    nc.tensor.matmul(
                        po[:], w2[:, hc], hT[:, hc],
                        start=(hc == 0), stop=(hc == HC - 1),
                    )
                nc.vector.tensor_add(resid[:, oc], resid[:, oc], po[:])

            nc.sync.dma_start(outT[:], resid[:])

    _split_multiwait(nc)
    return nc


def _prep_inputs(inputs):
    """Full problem inputs -> list of 8 per-core in_maps."""
    tgt = np.asarray(inputs["tgt"], np.float32)
    src = np.asarray(inputs["src"], np.float32)
    tgt_pos = np.asarray(inputs["tgt_pos"], np.int32)
    src_pos = np.asarray(inputs["src_pos"], np.int32)

    pre_ca_w = np.asarray(inputs["pre_ca_w"], np.float32)
    pre_sa_w = np.asarray(inputs["pre_sa_w"], np.float32)
    pre_ffn_w = np.asarray(inputs["pre_ffn_w"], np.float32)

    def fold(Wname, w):
        return np.asarray(inputs[Wname], np.float32) * w[:, None]

    ca_Wq = fold("ca_Wq", pre_ca_w)
    ca_Wkv = np.asarray(inputs["ca_Wkv"], np.float32)
    ca_Wk, ca_Wv = ca_Wkv[:, :DIM], ca_Wkv[:, DIM:]
    ca_Wo = np.asarray(inputs["ca_Wo"], np.float32)
    sa_Wq = fold("sa_Wq", pre_sa_w)
    sa_Wkv = fold("sa_Wkv", pre_sa_w)
    sa_Wk, sa_Wv = sa_Wkv[:, :DIM], sa_Wkv[:, DIM:]
    sa_Wo = np.asarray(inputs["sa_Wo"], np.float32)
    W1 = fold("W1", pre_ffn_w)
    W3 = fold("W3", pre_ffn_w)
    W2 = np.asarray(inputs["W2"], np.float32)

    shared = {
        "caWq": _lhsT_layout(ca_Wq),
        "caWk": _lhsT_layout(ca_Wk),
        "caWv": _rhs_layout(ca_Wv),
        "caWo": _lhsT_layout(ca_Wo),
        "saWq": _lhsT_layout(sa_Wq),
        "saWk": _lhsT_layout(sa_Wk),
        "saWv": _rhs_layout(sa_Wv),
        "saWo": _lhsT_layout(sa_Wo),
        "W1": _lhsT_layout(W1),
        "W3": _lhsT_layout(W3),
        "W2": _lhsT_layout(W2),
    }

    blk2 = np.zeros((P, 2), BF)
    blk2[:D, 0] = 1
    blk2[D:, 1] = 1
    shared["blk2"] = blk2

    def head_mask(w):  # [2, 128] with per-head norm weight
        m = np.zeros((2, P), np.float32)
        m[0, :D] = w
        m[1, D:] = w
        return m.astype(BF).copy()

    shared["mq_ca"] = head_mask(np.asarray(inputs["ca_qn"], np.float32))
    shared["mk_ca"] = head_mask(np.asarray(inputs["ca_kn"], np.float32))
    shared["mq_sa"] = head_mask(np.asarray(inputs["sa_qn"], np.float32))
    shared["mk_sa"] = head_mask(np.asarray(inputs["sa_kn"], np.float32))

    r64 = np.zeros((D, D), np.float32)
    half = D // 2
    for j in range(half):
        r64[j, j + half] = -1.0  # rot[j] = -x[j+32]
        r64[j + half, j] = 1.0  # rot[j+32] = x[j]
    rt = r64.T  # lhsT (matmul computes lhsT.T @ rhs)
    rotm = np.zeros((P, P), np.float32)
    rotm[:D, :D] = rt
    rotm[D:, D:] = rt
    shared["rotm"] = rotm.astype(BF).copy()

    shared["ones_c"] = np.ones((P, 1), BF)
    shared["ones_r128"] = np.ones((1, P), BF)

    in_maps = []
    for c in range(NCORES):
        s, part = c // NR, c % NR
        rows = slice(part * TOK, (part + 1) * TOK)
        m = dict(shared)
        m["tgtT"] = _featmajor(tgt[s, rows])
        m["srcTb"] = _featmajor(src[s, rows]).astype(BF)
        cq, sq_ = _rope_tables(tgt_pos[s, rows])
        ck, sk = _rope_tables(src_pos[s, rows])
        m["cosq"], m["sinq"] = cq, sq_
        m["coskca"], m["sinkca"] = ck, sk
        in_maps.append(m)
    return in_maps


def _get_nc():
    if "nc" not in _cache:
        _cache["nc"] = _build_bass()
    return _cache["nc"]


def run(inputs, trace=False):
    """Run on 8 cores; returns (full_output, exec_time_ns_or_None)."""
    if trace:
        _install_ntff_hook()
    from concourse.bass_utils import run_bass_kernel_spmd

    in_maps = _prep_inputs(inputs)
    nc = _get_nc()
    res = run_bass_kernel_spmd(
        nc, in_maps, core_ids=list(range(NCORES)), trace=trace
    )
    out = np.empty((B, N, DIM), np.float32)
    for c in range(NCORES):
        s, part = c // NR, c % NR
        arr = np.asarray(res.results[c]["outT"])  # [128, 8, TOK]
        rows = slice(part * TOK, (part + 1) * TOK)
        out[s, rows] = np.transpose(arr, (2, 1, 0)).reshape(TOK, DIM)
    return out, res.exec_time_ns


def kernel(**inputs):
    out, _ = run(inputs, trace=False)
    return out



# revision 14
# speedup vs baseline: 1.3285x; 1.3285x over previous
"""Trainium2 Bass kernel for nn_CrossLayer (dense transformer layer).

Sharding: sequence-parallel over 8 cores (2 samples x 4 token-chunks of 512).
Each core computes its 512 token rows through CA -> SA -> FFN. K/V for all 16
heads are computed from each core's own rows and AllGather'd (fp8) across the
4 cores of its sample, once per attention block.

On-chip layout: activations feature-major [dim(128p x 8c), tok] so every
matmul contracts over partitions. RMSNorm partition-sums via ones-matmuls on
PE; RoPE rotate-half via a constant +-1 block matrix on PE; softmax
denominators ride in an appended ones column on V, are broadcast across
partitions with a rank-1 matmul, and inverted with 128-lane Ln/Exp; exp
without max subtraction (scores are O(1): q/k are rms-normalized and
/sqrt(d)). K/V/Q and attention probabilities are fp8e4m3 (validated: adds
~6e-4 rel err on top of the bf16 baseline's ~1e-3, tolerance is 2e-2).
"""

import math
import sys
import types

import numpy as np
import ml_dtypes

B, N, DIM, HID, H, D = 2, 2048, 1024, 4096, 16, 64
TOK = 512  # tokens per core
NCORES = 8
EPS = 1e-6
THETA = 10000.0
P = 128
KO = DIM // P  # 8 contraction chunks
HH = H // 2  # 8 head pairs
HC = HID // P  # 32 hidden chunks
TC = TOK // P  # 4 token chunks per core
NR = 4  # ranks per replica group
VW = D + 1  # v columns + ones column

BF = ml_dtypes.bfloat16
F8 = ml_dtypes.float8_e4m3

_cache = {}


def _grouped_lhsT(W, G):
    """[K, M] -> [G, P, M//(G*P), K//P, P]: slice [g] loads contiguous and
    gives matmul lhsT tiles [128(K%128), j, c, 128(M%128)]."""
    K, M = W.shape
    J = M // (G * P)
    # arr[g, kp, j, c, mp] = W[c*P+kp, (g*J+j)*P+mp]
    return (
        W.reshape(K // P, P, G, J, P)
        .transpose(2, 1, 3, 0, 4)
        .astype(BF)
        .copy()
    )


def _vrhs_layout(W):
    """[K, M=DIM] -> [2, P, K//P, TOK]: slice [nh] is the rhs for v-feature
    half nh, contiguous."""
    K, M = W.shape
    # arr[nh, kp, c, m] = W[c*P+kp, nh*TOK+m]
    return W.reshape(K // P, P, 2, TOK).transpose(2, 1, 0, 3).astype(BF).copy()


def _w2_layout(W):
    """[HID, DIM] -> [KO, P, HC, P]: slice [oc] contiguous lhsT tiles."""
    # arr[oc, kp, hc, mp] = W[hc*P+kp, oc*P+mp]
    return (
        W.reshape(HC, P, KO, P).transpose(2, 1, 0, 3).astype(BF).copy()
    )


def _featmajor(x):
    """[tok, dim] -> [128, dim//128, tok] float32."""
    return x.T.reshape(DIM // P, P, x.shape[0]).transpose(1, 0, 2).copy()


def _rope_tables(pos):
    """pos [TOK] int32 -> cos/sin [128, TOK] (2 heads stacked) bf16."""
    invf = 1.0 / (THETA ** (np.arange(0, D, 2, dtype=np.float64) / D))  # [32]
    ang = pos.astype(np.float64)[None, :] * invf[:, None]  # [32, TOK]
    c = np.cos(ang)
    s = np.sin(ang)
    c64 = np.concatenate([c, c], axis=0)  # [64, TOK]
    s64 = np.concatenate([s, s], axis=0)
    c128 = np.concatenate([c64, c64], axis=0).astype(BF)  # [128, TOK]
    s128 = np.concatenate([s64, s64], axis=0).astype(BF)
    return c128.copy(), s128.copy()


def _install_ntff_hook():
    try:
        from trn_agent_boot.trn_boot import _ntff_profile_via_ctypes
    except ImportError:
        return
    if "antenv.axon_hooks" in sys.modules:
        return
    try:
        hook = _ntff_profile_via_ctypes("/opt/axon/libaxon_pjrt.so")
    except OSError:
        return
    mod = types.ModuleType("antenv.axon_hooks")
    mod.get_axon_ntff_profile_hook = lambda: hook
    mod.set_axon_ntff_profile_hook = lambda h: None
    sys.modules["antenv.axon_hooks"] = mod
    import antenv

    antenv.axon_hooks = mod


def _split_multiwait(nc):
    """This walrus only supports one sync-wait on CTRL-encoded instructions
    (Drain/NoOp); hoist excess waits onto single-wait NoOps placed before."""
    from concourse import mybir

    n_split = 0
    for f in nc.m.functions:
        for bb in f.blocks:
            new = []
            changed = False
            for ins in bb.instructions:
                si = ins.sync_info
                if (
                    si is not None
                    and si.on_wait is not None
                    and len(si.on_wait) > 1
                ):
                    waits = list(si.on_wait)
                    keep, rest = waits[:1], waits[1:]
                    for k, w in enumerate(rest):
                        new.append(
                            mybir.InstNoOp(
                                name=f"{ins.name}-wsplit{k}",
                                engine=ins.engine,
                                sync_info=mybir.SyncInfo(
                                    on_wait=[w], on_update=[]
                                ),
                                bass_nofuse=True,
                            )
                        )
                    si.on_wait = keep
                    n_split += 1
                    changed = True
                new.append(ins)
            if changed:
                bb.instructions = new
    return n_split


def _build_bass():
    from contextlib import ExitStack

    import concourse.bass as bass
    import concourse.tile as tile
    from concourse import mybir

    f32 = mybir.dt.float32
    bf16 = mybir.dt.bfloat16
    fp8 = mybir.dt.float8e4
    AF = mybir.ActivationFunctionType

    nc = bass.Bass(num_devices=NCORES)

    def inp(name, shape, dt=bf16):
        return nc.dram_tensor(name, shape, dt, kind="ExternalInput")

    tgtT = inp("tgtT", [P, KO, TOK], f32)
    srcTb = inp("srcTb", [P, KO, TOK])
    cosq = inp("cosq", [P, TOK])
    sinq = inp("sinq", [P, TOK])
    coskca = inp("coskca", [P, TOK])
    sinkca = inp("sinkca", [P, TOK])
    caWq = inp("caWq", [4, P, 2, KO, P])
    caWk = inp("caWk", [4, P, 2, KO, P])
    caWv = inp("caWv", [2, P, KO, TOK])
    caWo = inp("caWo", [4, P, 2, KO, P])
    saWq = inp("saWq", [4, P, 2, KO, P])
    saWk = inp("saWk", [4, P, 2, KO, P])
    saWv = inp("saWv", [2, P, KO, TOK])
    saWo = inp("saWo", [4, P, 2, KO, P])
    W1i = inp("W1", [16, P, 2, KO, P])  # half-group granularity
    W3i = inp("W3", [16, P, 2, KO, P])
    W2i = inp("W2", [KO, 2, P, HC // 2, P])  # half-oc granularity
    blk2 = inp("blk2", [P, 2])  # per-head ssq lhsT (block ones)
    mq_ca = inp("mq_ca", [2, P])  # rsqrt bcast lhsT with qn folded
    mk_ca = inp("mk_ca", [2, P])
    mq_sa = inp("mq_sa", [2, P])
    mk_sa = inp("mk_sa", [2, P])
    rotm = inp("rotm", [P, P])  # rotate-half (2-head block diag) lhsT
    ones_c = inp("ones_c", [P, 1])  # y-norm ssq lhsT
    ones_r128 = inp("ones_r128", [1, P])  # bcast lhsT (also den bcast)

    outT = nc.dram_tensor("outT", [P, KO, TOK], f32, kind="ExternalOutput")

    groups = [[0, 1, 2, 3], [4, 5, 6, 7]]
    KWORDS = P * HH * TOK  # k fp8 bytes per rank
    VWORDS = P * TC * H * VW  # v fp8 bytes per rank

    with tile.TileContext(nc) as tc:
        ctx = ExitStack()
        with ctx:
            sing = ctx.enter_context(tc.tile_pool(name="sing", bufs=1))
            wpool = ctx.enter_context(tc.tile_pool(name="wpool", bufs=2))
            work = ctx.enter_context(tc.tile_pool(name="work", bufs=2))
            probp = ctx.enter_context(tc.tile_pool(name="probp", bufs=2))
            kvpool = ctx.enter_context(tc.tile_pool(name="kvpool", bufs=1))
            dram = ctx.enter_context(
                tc.tile_pool(name="dram", bufs=2, space="DRAM")
            )
            pp = ctx.enter_context(tc.tile_pool(name="pp", bufs=2, space="PSUM"))
            ps_s = ctx.enter_context(
                tc.tile_pool(name="ps_s", bufs=2, space="PSUM")
            )
            ps_x = ctx.enter_context(
                tc.tile_pool(name="ps_x", bufs=1, space="PSUM")
            )

            # ---- resident tiles
            rotm_sb = sing.tile([P, P], bf16)
            nc.sync.dma_start(rotm_sb[:], rotm[:])
            resid = sing.tile([P, KO, TOK], f32)
            nc.sync.dma_start(resid[:], tgtT[:])
            srcT_sb = kvpool.tile([P, KO, TOK], bf16, tag="big", name="srcT_sb")
            nc.sync.dma_start(srcT_sb[:], srcTb[:])
            cosq_sb = sing.tile([P, TOK], bf16)
            nc.sync.dma_start(cosq_sb[:], cosq[:])
            sinq_sb = sing.tile([P, TOK], bf16)
            nc.sync.dma_start(sinq_sb[:], sinq[:])
            coskca_sb = sing.tile([P, TOK], bf16)
            nc.sync.dma_start(coskca_sb[:], coskca[:])
            sinkca_sb = sing.tile([P, TOK], bf16)
            nc.sync.dma_start(sinkca_sb[:], sinkca[:])
            blk2_sb = sing.tile([P, 2], bf16)
            nc.sync.dma_start(blk2_sb[:], blk2[:])
            masks_sb = {}
            for name, t in (
                ("mq_ca", mq_ca),
                ("mk_ca", mk_ca),
                ("mq_sa", mq_sa),
                ("mk_sa", mk_sa),
            ):
                m = sing.tile([2, P], bf16, name=name)
                nc.sync.dma_start(m[:], t[:])
                masks_sb[name] = m
            ones_c_sb = sing.tile([P, 1], bf16)
            nc.sync.dma_start(ones_c_sb[:], ones_c[:])
            ones_r128_sb = sing.tile([1, P], bf16)
            nc.sync.dma_start(ones_r128_sb[:], ones_r128[:])
            eps_sb = sing.tile([2, 1], mybir.dt.float32)
            nc.vector.memset(eps_sb[:], float(EPS))
            # all-ones [65, P]; row 64 is the den-broadcast lhsT (base
            # partition 64 matches the px ones-column row)
            ones64_sb = sing.tile([VW, P], bf16)
            nc.vector.memset(ones64_sb[:], 1.0)

            # ---- PE warm-up: ~3.5us of junk matmuls so HAM unthrottles
            # before the first real projection stream (output never read).
            junk_ps = pp.tile([P, P], f32, tag="pp", name="junk_ps")
            for _ in range(52):
                nc.tensor.matmul(
                    junk_ps[:, 0:64],
                    rotm_sb[:],
                    rotm_sb[:, 0:64],
                    start=True,
                    stop=True,
                )

            WAVE = 4  # head-pairs per norm/rope wave (bounds live buffers)

            def norm_rope_one(pk, uid):
                """Phase 1, per hh: pull raw + squared copies out of PSUM."""
                raw = work.tile([P, TOK], bf16, tag="raw", bufs=WAVE,
                                name=f"raw{uid}")
                nc.vector.tensor_copy(raw[:], pk[:])
                sq = work.tile([P, TOK], bf16, tag="sq", bufs=WAVE,
                               name=f"sq{uid}")
                nc.scalar.activation(sq[:], pk[:], AF.Square)
                return raw, sq

            def norm_rope_two(raw, sq, uid):
                ssq = ps_s.tile([2, TOK], f32, tag="ps", name=f"ssq{uid}")
                nc.tensor.matmul(ssq[:], blk2_sb[:], sq[:], start=True, stop=True)
                rot_ps = pp.tile([P, TOK], f32, tag="pp", name=f"rotp{uid}")
                nc.tensor.matmul(rot_ps[:], rotm_sb[:], raw[:], start=True,
                                 stop=True)
                rot = work.tile([P, TOK], bf16, tag="rot", bufs=WAVE,
                                name=f"rot{uid}")
                nc.vector.tensor_copy(rot[:], rot_ps[:])
                return ssq, rot

            def norm_rope_three(ssq, uid):
                # rsqrt(mean+eps) = exp(-0.5*ln(mean+eps)); Ln/Exp live in
                # the same ACT table set as the attention exps.
                lnt = work.tile([2, TOK], bf16, tag="lnt", name=f"lnt{uid}")
                nc.scalar.activation(
                    lnt[:], ssq[:], AF.Ln, bias=eps_sb[:], scale=1.0 / D
                )
                rs = work.tile([2, TOK], bf16, tag="rs", name=f"rs{uid}")
                nc.scalar.activation(rs[:], lnt[:], AF.Exp, scale=-0.5)
                return rs

            def norm_rope_four(raw, rot, rs, mask_sb, cos_sb, sin_sb, dst,
                               uid):
                bc = pp.tile([P, TOK], f32, tag="pp", name=f"bc{uid}")
                nc.tensor.matmul(bc[:], mask_sb[:], rs[:], start=True, stop=True)
                t1 = work.tile([P, TOK], bf16, tag="t1", name=f"t1{uid}")
                nc.vector.tensor_mul(t1[:], raw[:], cos_sb[:])
                t2 = work.tile([P, TOK], bf16, tag="t2", name=f"t2{uid}")
                nc.vector.tensor_mul(t2[:], rot[:], sin_sb[:])
                u = work.tile([P, TOK], bf16, tag="u", name=f"u{uid}")
                nc.vector.tensor_add(u[:], t1[:], t2[:])
                nc.vector.tensor_mul(dst, u[:], bc[:])

            def norm_wave(pks, mask, cos_sb, sin_sb, dstf, hhs, uid):
                """Norm+rope a wave of head-pair psums, phase-batched so
                each engine's stream stays dense and buffer lifetimes stay
                within the wave."""
                raws, sqs, ssqs, rots, rss = {}, {}, {}, {}, {}
                for hh in hhs:
                    raws[hh], sqs[hh] = norm_rope_one(pks[hh], f"{uid}{hh}")
                for hh in hhs:
                    ssqs[hh], rots[hh] = norm_rope_two(
                        raws[hh], sqs[hh], f"{uid}{hh}"
                    )
                for hh in hhs:
                    rss[hh] = norm_rope_three(ssqs[hh], f"{uid}{hh}")
                for hh in hhs:
                    norm_rope_four(
                        raws[hh], rots[hh], rss[hh], mask, cos_sb, sin_sb,
                        dstf(hh), f"{uid}{hh}",
                    )

            def proj_norm_block(kvsrc_sb, W_t, mask, cos_sb, sin_sb, dstf,
                                uid, wtag="wkq"):
                """Project 16 heads from kvsrc and norm+rope them; dstf(hh)
                gives the destination AP per head-pair."""
                for wave in range(HH // WAVE):
                    pks = {}
                    for g in range(wave * 2, wave * 2 + 2):
                        wk = wpool.tile([P, 2, KO, P], bf16, tag=wtag,
                                        name=f"wk{uid}{g}")
                        nc.sync.dma_start(wk[:], W_t[g])
                        for j in range(2):
                            hh = g * 2 + j
                            pk = pp.tile([P, TOK], f32, tag="pp",
                                         name=f"pk{uid}{hh}")
                            for c in range(KO):
                                nc.tensor.matmul(
                                    pk[:],
                                    wk[:, j, c],
                                    kvsrc_sb[:, c],
                                    start=(c == 0),
                                    stop=(c == KO - 1),
                                )
                            pks[hh] = pk
                    hhs = range(wave * WAVE, (wave + 1) * WAVE)
                    norm_wave(pks, mask, cos_sb, sin_sb, dstf, hhs, uid)

            def rmsnorm_feat(src_f32, dst_bf16, uid):
                """Feature-major RMSNorm: dst = src * rsqrt(mean(src^2))."""
                ssq = ps_s.tile([1, TOK], f32, tag="ps", name=f"yssq{uid}")
                for c in range(KO):
                    ysq = work.tile([P, TOK], bf16, tag="ysq",
                                    name=f"ysq{uid}{c}")
                    nc.vector.tensor_mul(ysq[:], src_f32[:, c], src_f32[:, c])
                    nc.tensor.matmul(
                        ssq[:],
                        ones_c_sb[:],
                        ysq[:],
                        start=(c == 0),
                        stop=(c == KO - 1),
                    )
                lnt = work.tile([1, TOK], f32, tag="lnt", name=f"ylnt{uid}")
                nc.scalar.activation(
                    lnt[:], ssq[:], AF.Ln, bias=eps_sb[:1], scale=1.0 / DIM
                )
                rs = work.tile([1, TOK], bf16, tag="rs", name=f"yrs{uid}")
                nc.scalar.activation(rs[:], lnt[:], AF.Exp, scale=-0.5)
                bc = pp.tile([P, TOK], f32, tag="pp", name=f"ybc{uid}")
                nc.tensor.matmul(
                    bc[:], ones_r128_sb[:], rs[:], start=True, stop=True
                )
                for c in range(KO):
                    nc.vector.tensor_mul(dst_bf16[:, c], src_f32[:, c], bc[:])

            def attention_block(y_fn, kvsrc_fn, Wq_t, Wk_t, Wv_t, Wo_t,
                                mq, mk, cosk, sink, uid, prefetch=None):
                """One attention block. y_fn()/kvsrc_fn() return the q-side /
                kv-side SBUF inputs (y is computed lazily so the kv side can
                be projected and gathered first). Adds Wo output to resid."""
                kvsrc_sb = kvsrc_fn()

                # --- k projection + norm/rope from my rows (fp8 out)
                k_mine = kvpool.tile([P, HH, TOK], fp8, tag="kmine",
                                     name=f"k_mine{uid}")
                proj_norm_block(
                    kvsrc_sb, Wk_t, mk, cosk, sink,
                    lambda hh: k_mine[:, hh], f"k{uid}",
                )

                # --- v projection (token-major, with ones column, fp8)
                v_mine = kvpool.tile(
                    [P, TC, H, VW], fp8, tag="vmine", name=f"v_mine{uid}"
                )
                nc.vector.memset(v_mine[:, :, :, D : D + 1], 1.0)
                for nh in range(2):
                    wv = wpool.tile([P, KO, TOK], bf16, tag="wv", bufs=1,
                                    name=f"wv{uid}{nh}")
                    nc.sync.dma_start(wv[:], Wv_t[nh])
                    for t4 in range(TC):
                        pv = pp.tile([P, TOK], f32, tag="pp",
                                     name=f"pv{uid}{nh}{t4}")
                        for c in range(KO):
                            nc.tensor.matmul(
                                pv[:],
                                kvsrc_sb[:, c, t4 * P : (t4 + 1) * P],
                                wv[:, c],
                                start=(c == 0),
                                stop=(c == KO - 1),
                            )
                        nc.vector.tensor_copy(
                            v_mine[:, t4, nh * 8 : (nh + 1) * 8, 0:D],
                            pv[:].rearrange("p (h d) -> p h d", d=D),
                        )

                # --- allgather k/v across my sample's 4 cores (fp8)
                kv_in = dram.tile([KWORDS + VWORDS], fp8, tag="kv_in",
                                  name=f"kv_in{uid}")
                nc.sync.dma_start(
                    kv_in[:KWORDS].rearrange(
                        "(p h t) -> p h t", p=P, h=HH, t=TOK
                    ),
                    k_mine[:],
                )
                nc.sync.dma_start(
                    kv_in[KWORDS:].rearrange(
                        "(p a b c) -> p a b c", p=P, a=TC, b=H, c=VW
                    ),
                    v_mine[:],
                )
                kv_out = dram.tile([NR, KWORDS + VWORDS], fp8, tag="kv_out",
                                   name=f"kv_out{uid}")
                nc.gpsimd.collective_compute(
                    "AllGather",
                    mybir.AluOpType.bypass,
                    replica_groups=groups,
                    ins=[kv_in.opt()],
                    outs=[kv_out.opt()],
                )

                # --- wo loads + optional FFN prefetch go on the DMA queue
                # BEFORE the gather-dependent readbacks (SP DMA is FIFO).
                wo_tiles = []
                for g in range(4):
                    wo = wpool.tile([P, 2, KO, P], bf16, tag="wo",
                                    name=f"wo{uid}{g}")
                    nc.sync.dma_start(wo[:], Wo_t[g])
                    wo_tiles.append(wo)
                if prefetch is not None:
                    prefetch()

                # --- y-norm + q projection + norm + rope (overlap gather)
                y_sb = y_fn()
                q_sb = kvpool.tile([P, HH, TOK], fp8, tag="q",
                                   name=f"q_sb{uid}")
                proj_norm_block(
                    y_sb, Wq_t, mq, cosq_sb, sinq_sb,
                    lambda hh: q_sb[:, hh], f"q{uid}",
                )

                # --- gather readback (after all independent DMAs)
                k_full = kvpool.tile(
                    [P, NR, HH, TOK], fp8, tag="kfull", name=f"k_full{uid}"
                )
                v_full = kvpool.tile(
                    [P, NR, TC, H, VW], fp8, tag="vfull", name=f"v_full{uid}"
                )
                for r in range(NR):
                    nc.sync.dma_start(
                        k_full[:, r],
                        kv_out[r, :KWORDS].rearrange(
                            "(p h t) -> p h t", p=P, h=HH, t=TOK
                        ),
                    )
                    nc.sync.dma_start(
                        v_full[:, r],
                        kv_out[r, KWORDS:].rearrange(
                            "(p a b c) -> p a b c", p=P, a=TC, b=H, c=VW
                        ),
                    )

                # --- attention: 2 heads share one exp; denominators ride in
                # row 64 of the px accumulators (ones column of v)
                xT = kvpool.tile([P, HH, TOK], bf16, tag="xT", name=f"xT{uid}")
                for hh in range(HH):
                    px = [
                        ps_x.tile([VW, TOK], f32, tag=f"px{i}",
                                  name=f"px{uid}{hh}{i}")
                        for i in range(2)
                    ]
                    probs = {}
                    # software-pipelined: scores run one chunk ahead of px
                    def scores(kc):
                        r, tcl = kc // TC, kc % TC
                        ps = ps_s.tile([P, 2 * TOK], f32, tag="ps",
                                       name=f"ps{uid}{hh}{kc}")
                        for i in range(2):
                            off = i * D
                            nc.tensor.matmul(
                                ps[:, i * TOK : (i + 1) * TOK],
                                k_full[
                                    off : off + D, r, hh,
                                    tcl * P : (tcl + 1) * P,
                                ],
                                q_sb[off : off + D, hh],
                                start=True,
                                stop=True,
                            )
                        prob = probp.tile([P, 2 * TOK], fp8, tag="prob",
                                          name=f"prob{uid}{hh}{kc}")
                        nc.scalar.activation(
                            prob[:], ps[:], AF.Exp, scale=1.0 / math.sqrt(D)
                        )
                        probs[kc] = prob

                    def pxacc(kc):
                        r, tcl = kc // TC, kc % TC
                        for i in range(2):
                            h = hh * 2 + i
                            nc.tensor.matmul(
                                px[i][:],
                                v_full[:, r, tcl, h],
                                probs[kc][:, i * TOK : (i + 1) * TOK],
                                start=(kc == 0),
                                stop=(kc == H - 1),
                            )

                    scores(0)
                    for kc in range(1, H):
                        scores(kc)
                        pxacc(kc - 1)
                    pxacc(H - 1)

                    # denominators: stay on partition 64, broadcast to all
                    # 128 partitions via a rank-1 matmul (lhsT/rhs both at
                    # base partition 64), invert with 128-lane Ln/Exp.
                    dsb = work.tile([VW, 2, TOK], bf16, tag="dsb", bufs=1,
                                    name=f"dsb{uid}{hh}")
                    for i in range(2):
                        nc.vector.tensor_copy(
                            dsb[D : D + 1, i], px[i][D : D + 1]
                        )
                        nc.vector.tensor_copy(
                            xT[i * D : (i + 1) * D, hh], px[i][0:D]
                        )
                    dbc = ps_s.tile([P, 2 * TOK], f32, tag="ps",
                                    name=f"dbc{uid}{hh}")
                    for i in range(2):
                        nc.tensor.matmul(
                            dbc[:, i * TOK : (i + 1) * TOK],
                            ones64_sb[D : D + 1, :],
                            dsb[D : D + 1, i],
                            start=True,
                            stop=True,
                        )
                    dln = work.tile([P, 2 * TOK], f32, tag="dln", bufs=1,
                                    name=f"dln{uid}{hh}")
                    nc.scalar.activation(dln[:], dbc[:], AF.Ln)
                    rec = work.tile([P, 2 * TOK], bf16, tag="rec",
                                    name=f"rec{uid}{hh}")
                    nc.scalar.activation(rec[:], dln[:], AF.Exp, scale=-1.0)
                    for i in range(2):
                        sl = slice(i * D, (i + 1) * D)
                        nc.vector.tensor_mul(
                            xT[sl, hh], xT[sl, hh],
                            rec[sl, i * TOK : (i + 1) * TOK],
                        )

                # --- Wo projection, accumulate into resid
                for g in range(4):
                    for j in range(2):
                        oc = g * 2 + j
                        po = pp.tile([P, TOK], f32, tag="pp",
                                     name=f"po{uid}{oc}")
                        for c in range(KO):
                            nc.tensor.matmul(
                                po[:],
                                wo_tiles[g][:, j, c],
                                xT[:, c],
                                start=(c == 0),
                                stop=(c == KO - 1),
                            )
                        nc.vector.tensor_add(resid[:, oc], resid[:, oc], po[:])

            # ================= cross-attention =================
            yT = sing.tile([P, KO, TOK], bf16, name="yT")

            def ca_y():
                rmsnorm_feat(resid, yT, "ca")
                return yT

            attention_block(
                ca_y, lambda: srcT_sb, caWq, caWk, caWv, caWo,
                masks_sb["mq_ca"], masks_sb["mk_ca"], coskca_sb, sinkca_sb,
                "ca",
            )

            # ================= self-attention =================
            ffn_pre = {}

            def ffn_prefetch():
                for g in range(2):
                    w1 = wpool.tile([P, 2, KO, P], bf16, tag="w1",
                                    name=f"w1p{g}")
                    nc.sync.dma_start(w1[:], W1i[g])
                    w3 = wpool.tile([P, 2, KO, P], bf16, tag="w3",
                                    name=f"w3p{g}")
                    nc.sync.dma_start(w3[:], W3i[g])
                    ffn_pre[g] = (w1, w3)

            def sa_y():
                rmsnorm_feat(resid, yT, "sa")
                return yT

            sa_y_done = {}

            def sa_y_once():
                if "y" not in sa_y_done:
                    sa_y_done["y"] = sa_y()
                return sa_y_done["y"]

            attention_block(
                sa_y_once, sa_y_once, saWq, saWk, saWv, saWo,
                masks_sb["mq_sa"], masks_sb["mk_sa"], cosq_sb, sinq_sb,
                "sa", prefetch=ffn_prefetch,
            )

            # ================= FFN =================
            rmsnorm_feat(resid, yT, "ffn")
            hT = kvpool.tile([P, HC, TOK], bf16, tag="big", name="hT")
            for gh in range(16):  # half-groups of 2 output chunks
                if gh in ffn_pre:
                    w1, w3 = ffn_pre[gh]
                else:
                    w1 = wpool.tile([P, 2, KO, P], bf16, tag="w1",
                                    name=f"w1{gh}")
                    nc.sync.dma_start(w1[:], W1i[gh])
                    w3 = wpool.tile([P, 2, KO, P], bf16, tag="w3",
                                    name=f"w3{gh}")
                    nc.sync.dma_start(w3[:], W3i[gh])
                for j in range(2):
                    hc = gh * 2 + j
                    p1 = pp.tile([P, TOK], f32, tag="pp", name=f"p1{hc}")
                    for c in range(KO):
                        nc.tensor.matmul(
                            p1[:], w1[:, j, c], yT[:, c],
                            start=(c == 0), stop=(c == KO - 1),
                        )
                    p3 = pp.tile([P, TOK], f32, tag="pp", name=f"p3{hc}")
                    for c in range(KO):
                        nc.tensor.matmul(
                            p3[:], w3[:, j, c], yT[:, c],
                            start=(c == 0), stop=(c == KO - 1),
                        )
                    s1 = work.tile([P, TOK], bf16, tag="s1", name=f"s1{hc}")
                    nc.scalar.activation(s1[:], p1[:], AF.Silu)
                    nc.vector.tensor_mul(hT[:, hc], s1[:], p3[:])
            for oc in range(KO):
                po = ps_s.tile([P, TOK], f32, tag="ps", name=f"po2{oc}")
                for half in range(2):
                    w2 = wpool.tile([P, HC // 2, P], bf16, tag="w2",
                                    name=f"w2{oc}{half}")
                    nc.sync.dma_start(w2[:], W2i[oc, half])
                    for hc in range(HC // 2):
                        hca = half * (HC // 2) + hc
                        nc.tensor.matmul(
                            po[:], w2[:, hc], hT[:, hca],
                            start=(hca == 0), stop=(hca == HC - 1),
                        )
                nc.vector.tensor_add(resid[:, oc], resid[:, oc], po[:])

            nc.sync.dma_start(outT[:], resid[:])

    _split_multiwait(nc)
    return nc


def _prep_inputs(inputs):
    """Full problem inputs -> list of 8 per-core in_maps."""
    tgt = np.asarray(inputs["tgt"], np.float32)
    src = np.asarray(inputs["src"], np.float32)
    tgt_pos = np.asarray(inputs["tgt_pos"], np.int32)
    src_pos = np.asarray(inputs["src_pos"], np.int32)

    pre_ca_w = np.asarray(inputs["pre_ca_w"], np.float32)
    pre_sa_w = np.asarray(inputs["pre_sa_w"], np.float32)
    pre_ffn_w = np.asarray(inputs["pre_ffn_w"], np.float32)

    def fold(Wname, w):
        return np.asarray(inputs[Wname], np.float32) * w[:, None]

    ca_Wq = fold("ca_Wq", pre_ca_w)
    ca_Wkv = np.asarray(inputs["ca_Wkv"], np.float32)
    ca_Wk, ca_Wv = ca_Wkv[:, :DIM], ca_Wkv[:, DIM:]
    ca_Wo = np.asarray(inputs["ca_Wo"], np.float32)
    sa_Wq = fold("sa_Wq", pre_sa_w)
    sa_Wkv = fold("sa_Wkv", pre_sa_w)
    sa_Wk, sa_Wv = sa_Wkv[:, :DIM], sa_Wkv[:, DIM:]
    sa_Wo = np.asarray(inputs["sa_Wo"], np.float32)
    W1 = fold("W1", pre_ffn_w)
    W3 = fold("W3", pre_ffn_w)
    W2 = np.asarray(inputs["W2"], np.float32)

    shared = {
        "caWq": _grouped_lhsT(ca_Wq, 4),
        "caWk": _grouped_lhsT(ca_Wk, 4),
        "caWv": _vrhs_layout(ca_Wv),
        "caWo": _grouped_lhsT(ca_Wo, 4),
        "saWq": _grouped_lhsT(sa_Wq, 4),
        "saWk": _grouped_lhsT(sa_Wk, 4),
        "saWv": _vrhs_layout(sa_Wv),
        "saWo": _grouped_lhsT(sa_Wo, 4),
        "W1": _grouped_lhsT(W1, 16),
        "W3": _grouped_lhsT(W3, 16),
        "W2": _w2_layout(W2).reshape(KO, P, 2, HC // 2, P).transpose(
            0, 2, 1, 3, 4).copy(),
    }

    blk2 = np.zeros((P, 2), BF)
    blk2[:D, 0] = 1
    blk2[D:, 1] = 1
    shared["blk2"] = blk2

    def head_mask(w):  # [2, 128] with per-head norm weight
        m = np.zeros((2, P), np.float32)
        m[0, :D] = w
        m[1, D:] = w
        return m.astype(BF).copy()

    shared["mq_ca"] = head_mask(np.asarray(inputs["ca_qn"], np.float32))
    shared["mk_ca"] = head_mask(np.asarray(inputs["ca_kn"], np.float32))
    shared["mq_sa"] = head_mask(np.asarray(inputs["sa_qn"], np.float32))
    shared["mk_sa"] = head_mask(np.asarray(inputs["sa_kn"], np.float32))

    r64 = np.zeros((D, D), np.float32)
    half = D // 2
    for j in range(half):
        r64[j, j + half] = -1.0  # rot[j] = -x[j+32]
        r64[j + half, j] = 1.0  # rot[j+32] = x[j]
    rt = r64.T  # lhsT (matmul computes lhsT.T @ rhs)
    rotm = np.zeros((P, P), np.float32)
    rotm[:D, :D] = rt
    rotm[D:, D:] = rt
    shared["rotm"] = rotm.astype(BF).copy()

    shared["ones_c"] = np.ones((P, 1), BF)
    shared["ones_r128"] = np.ones((1, P), BF)

    in_maps = []
    for c in range(NCORES):
        s, part = c // NR, c % NR
        rows = slice(part * TOK, (part + 1) * TOK)
        m = dict(shared)
        m["tgtT"] = _featmajor(tgt[s, rows])
        m["srcTb"] = _featmajor(src[s, rows]).astype(BF)
        cq, sq_ = _rope_tables(tgt_pos[s, rows])
        ck, sk = _rope_tables(src_pos[s, rows])
        m["cosq"], m["sinq"] = cq, sq_
        m["coskca"], m["sinkca"] = ck, sk
        in_maps.append(m)
    return in_maps


def _get_nc():
    if "nc" not in _cache:
        _cache["nc"] = _build_bass()
    return _cache["nc"]


def run(inputs, trace=False):
    """Run on 8 cores; returns (full_output, exec_time_ns_or_None)."""
    if trace:
        _install_ntff_hook()
    from concourse.bass_utils import run_bass_kernel_spmd

    in_maps = _prep_inputs(inputs)
    nc = _get_nc()
    res = run_bass_kernel_spmd(
        nc, in_maps, core_ids=list(range(NCORES)), trace=trace
    )
    out = np.empty((B, N, DIM), np.float32)
    for c in range(NCORES):
        s, part = c // NR, c % NR
        arr = np.asarray(res.results[c]["outT"])  # [128, 8, TOK]
        rows = slice(part * TOK, (part + 1) * TOK)
        out[s, rows] = np.transpose(arr, (2, 1, 0)).reshape(TOK, DIM)
    return out, res.exec_time_ns


def kernel(**inputs):
    out, _ = run(inputs, trace=False)
    return out


# revision 19
# speedup vs baseline: 1.3404x; 1.0090x over previous
"""Trainium2 Bass kernel for nn_CrossLayer (dense transformer layer).

Sharding: sequence-parallel over 8 cores (2 samples x 4 token-chunks of 512).
Each core computes its 512 token rows through CA -> SA -> FFN. K/V for all 16
heads are computed from each core's own rows and AllGather'd (fp8) across the
4 cores of its sample, once per attention block.

On-chip layout: activations feature-major [dim(128p x 8c), tok] so every
matmul contracts over partitions. RMSNorm partition-sums via ones-matmuls on
PE; RoPE rotate-half via a constant +-1 block matrix on PE; softmax
denominators ride in an appended ones column on V, are broadcast across
partitions with a rank-1 matmul, and inverted with 128-lane Ln/Exp; exp
without max subtraction (scores are O(1): q/k are rms-normalized and
/sqrt(d)). K/V/Q and attention probabilities are fp8e4m3 (validated: adds
~6e-4 rel err on top of the bf16 baseline's ~1e-3, tolerance is 2e-2).
"""

import math
import sys
import types

import numpy as np
import ml_dtypes

B, N, DIM, HID, H, D = 2, 2048, 1024, 4096, 16, 64
TOK = 512  # tokens per core
NCORES = 8
EPS = 1e-6
THETA = 10000.0
P = 128
KO = DIM // P  # 8 contraction chunks
HH = H // 2  # 8 head pairs
HC = HID // P  # 32 hidden chunks
TC = TOK // P  # 4 token chunks per core
NR = 4  # ranks per replica group
VW = D + 1  # v columns + ones column

BF = ml_dtypes.bfloat16
F8 = ml_dtypes.float8_e4m3

_cache = {}


def _grouped_lhsT(W, G):
    """[K, M] -> [G, P, M//(G*P), K//P, P]: slice [g] loads contiguous and
    gives matmul lhsT tiles [128(K%128), j, c, 128(M%128)]."""
    K, M = W.shape
    J = M // (G * P)
    # arr[g, kp, j, c, mp] = W[c*P+kp, (g*J+j)*P+mp]
    return (
        W.reshape(K // P, P, G, J, P)
        .transpose(2, 1, 3, 0, 4)
        .astype(BF)
        .copy()
    )


def _vrhs_layout(W):
    """[K, M=DIM] -> [2, P, K//P, TOK]: slice [nh] is the rhs for v-feature
    half nh, contiguous."""
    K, M = W.shape
    # arr[nh, kp, c, m] = W[c*P+kp, nh*TOK+m]
    return W.reshape(K // P, P, 2, TOK).transpose(2, 1, 0, 3).astype(BF).copy()


def _w2_layout(W):
    """[HID, DIM] -> [KO, P, HC, P]: slice [oc] contiguous lhsT tiles."""
    # arr[oc, kp, hc, mp] = W[hc*P+kp, oc*P+mp]
    return (
        W.reshape(HC, P, KO, P).transpose(2, 1, 0, 3).astype(BF).copy()
    )


def _featmajor(x):
    """[tok, dim] -> [128, dim//128, tok] float32."""
    return x.T.reshape(DIM // P, P, x.shape[0]).transpose(1, 0, 2).copy()


def _rope_tables(pos):
    """pos [TOK] int32 -> cos/sin [128, TOK] (2 heads stacked) bf16."""
    invf = 1.0 / (THETA ** (np.arange(0, D, 2, dtype=np.float64) / D))  # [32]
    ang = pos.astype(np.float64)[None, :] * invf[:, None]  # [32, TOK]
    c = np.cos(ang)
    s = np.sin(ang)
    c64 = np.concatenate([c, c], axis=0)  # [64, TOK]
    s64 = np.concatenate([s, s], axis=0)
    c128 = np.concatenate([c64, c64], axis=0).astype(BF)  # [128, TOK]
    s128 = np.concatenate([s64, s64], axis=0).astype(BF)
    return c128.copy(), s128.copy()


def _install_ntff_hook():
    try:
        from trn_agent_boot.trn_boot import _ntff_profile_via_ctypes
    except ImportError:
        return
    if "antenv.axon_hooks" in sys.modules:
        return
    try:
        hook = _ntff_profile_via_ctypes("/opt/axon/libaxon_pjrt.so")
    except OSError:
        return
    mod = types.ModuleType("antenv.axon_hooks")
    mod.get_axon_ntff_profile_hook = lambda: hook
    mod.set_axon_ntff_profile_hook = lambda h: None
    sys.modules["antenv.axon_hooks"] = mod
    import antenv

    antenv.axon_hooks = mod


def _split_multiwait(nc):
    """This walrus only supports one sync-wait on CTRL-encoded instructions
    (Drain/NoOp); hoist excess waits onto single-wait NoOps placed before."""
    from concourse import mybir

    n_split = 0
    for f in nc.m.functions:
        for bb in f.blocks:
            new = []
            changed = False
            for ins in bb.instructions:
                si = ins.sync_info
                if (
                    si is not None
                    and si.on_wait is not None
                    and len(si.on_wait) > 1
                ):
                    waits = list(si.on_wait)
                    keep, rest = waits[:1], waits[1:]
                    for k, w in enumerate(rest):
                        new.append(
                            mybir.InstNoOp(
                                name=f"{ins.name}-wsplit{k}",
                                engine=ins.engine,
                                sync_info=mybir.SyncInfo(
                                    on_wait=[w], on_update=[]
                                ),
                                bass_nofuse=True,
                            )
                        )
                    si.on_wait = keep
                    n_split += 1
                    changed = True
                new.append(ins)
            if changed:
                bb.instructions = new
    return n_split


def _build_bass():
    from contextlib import ExitStack

    import concourse.bass as bass
    import concourse.tile as tile
    from concourse import mybir

    f32 = mybir.dt.float32
    bf16 = mybir.dt.bfloat16
    fp8 = mybir.dt.float8e4
    AF = mybir.ActivationFunctionType

    nc = bass.Bass(num_devices=NCORES)

    def inp(name, shape, dt=bf16):
        return nc.dram_tensor(name, shape, dt, kind="ExternalInput")

    tgtT = inp("tgtT", [P, KO, TOK], f32)
    srcTb = inp("srcTb", [P, KO, TOK])
    cosq = inp("cosq", [P, TOK])
    sinq = inp("sinq", [P, TOK])
    coskca = inp("coskca", [P, TOK])
    sinkca = inp("sinkca", [P, TOK])
    caWq = inp("caWq", [4, P, 2, KO, P])
    caWk = inp("caWk", [4, P, 2, KO, P])
    caWv = inp("caWv", [2, P, KO, TOK])
    caWo = inp("caWo", [4, P, 2, KO, P])
    saWq = inp("saWq", [4, P, 2, KO, P])
    saWk = inp("saWk", [4, P, 2, KO, P])
    saWv = inp("saWv", [2, P, KO, TOK])
    saWo = inp("saWo", [4, P, 2, KO, P])
    W1i = inp("W1", [16, P, 2, KO, P])  # half-group granularity
    W3i = inp("W3", [16, P, 2, KO, P])
    W2i = inp("W2", [KO, 2, P, HC // 2, P])  # half-oc granularity
    blk2 = inp("blk2", [P, 2])  # per-head ssq lhsT (block ones)
    mq_ca = inp("mq_ca", [2, P])  # rsqrt bcast lhsT with qn folded
    mk_ca = inp("mk_ca", [2, P])
    mq_sa = inp("mq_sa", [2, P])
    mk_sa = inp("mk_sa", [2, P])
    rotm = inp("rotm", [P, P])  # rotate-half (2-head block diag) lhsT
    ones_c = inp("ones_c", [P, 1])  # y-norm ssq lhsT
    ones_r128 = inp("ones_r128", [1, P])  # bcast lhsT (also den bcast)

    outT = nc.dram_tensor("outT", [P, KO, TOK], f32, kind="ExternalOutput")

    groups = [[0, 1, 2, 3], [4, 5, 6, 7]]
    KWORDS = P * HH * TOK  # k fp8 bytes per rank
    VWORDS = P * TC * H * VW  # v fp8 bytes per rank

    with tile.TileContext(nc) as tc:
        ctx = ExitStack()
        with ctx:
            sing = ctx.enter_context(tc.tile_pool(name="sing", bufs=1))
            wpool = ctx.enter_context(tc.tile_pool(name="wpool", bufs=2))
            work = ctx.enter_context(tc.tile_pool(name="work", bufs=2))
            probp = ctx.enter_context(tc.tile_pool(name="probp", bufs=2))
            kvpool = ctx.enter_context(tc.tile_pool(name="kvpool", bufs=1))
            dram = ctx.enter_context(
                tc.tile_pool(name="dram", bufs=2, space="DRAM")
            )
            pp = ctx.enter_context(tc.tile_pool(name="pp", bufs=2, space="PSUM"))
            ps_s = ctx.enter_context(
                tc.tile_pool(name="ps_s", bufs=2, space="PSUM")
            )
            ps_x = ctx.enter_context(
                tc.tile_pool(name="ps_x", bufs=1, space="PSUM")
            )

            # ---- resident tiles (srcT first: CA k-proj waits on it)
            rotm_sb = sing.tile([P, P], bf16)
            nc.sync.dma_start(rotm_sb[:], rotm[:])
            srcT_sb = kvpool.tile([P, KO, TOK], bf16, tag="big", name="srcT_sb")
            nc.sync.dma_start(srcT_sb[:], srcTb[:])
            resid = sing.tile([P, KO, TOK], f32)
            nc.sync.dma_start(resid[:], tgtT[:])
            cosq_sb = sing.tile([P, TOK], bf16)
            nc.sync.dma_start(cosq_sb[:], cosq[:])
            sinq_sb = sing.tile([P, TOK], bf16)
            nc.sync.dma_start(sinq_sb[:], sinq[:])
            coskca_sb = sing.tile([P, TOK], bf16)
            nc.sync.dma_start(coskca_sb[:], coskca[:])
            sinkca_sb = sing.tile([P, TOK], bf16)
            nc.sync.dma_start(sinkca_sb[:], sinkca[:])
            blk2_sb = sing.tile([P, 2], bf16)
            nc.sync.dma_start(blk2_sb[:], blk2[:])
            masks_sb = {}
            for name, t in (
                ("mq_ca", mq_ca),
                ("mk_ca", mk_ca),
                ("mq_sa", mq_sa),
                ("mk_sa", mk_sa),
            ):
                m = sing.tile([2, P], bf16, name=name)
                nc.sync.dma_start(m[:], t[:])
                masks_sb[name] = m
            ones_c_sb = sing.tile([P, 1], bf16)
            nc.sync.dma_start(ones_c_sb[:], ones_c[:])
            ones_r128_sb = sing.tile([1, P], bf16)
            nc.sync.dma_start(ones_r128_sb[:], ones_r128[:])
            eps_sb = sing.tile([2, 1], mybir.dt.float32)
            nc.vector.memset(eps_sb[:], float(EPS))
            # all-ones [65, P]; row 64 is the den-broadcast lhsT (base
            # partition 64 matches the px ones-column row)
            ones64_sb = sing.tile([VW, P], bf16)
            nc.vector.memset(ones64_sb[:], 1.0)

            # ---- PE warm-up: ~3.5us of junk matmuls so HAM unthrottles
            # before the first real projection stream (output never read).
            junk_ps = pp.tile([P, P], f32, tag="pp", name="junk_ps")
            for _ in range(52):
                nc.tensor.matmul(
                    junk_ps[:, 0:64],
                    rotm_sb[:],
                    rotm_sb[:, 0:64],
                    start=True,
                    stop=True,
                )

            WAVE = 4  # head-pairs per norm/rope wave (bounds live buffers)

            def norm_rope_one(pk, uid):
                """Phase 1, per hh: pull raw + squared copies out of PSUM."""
                raw = work.tile([P, TOK], bf16, tag="raw", bufs=WAVE,
                                name=f"raw{uid}")
                nc.vector.tensor_copy(raw[:], pk[:])
                sq = work.tile([P, TOK], bf16, tag="sq", bufs=WAVE,
                               name=f"sq{uid}")
                nc.scalar.activation(sq[:], pk[:], AF.Square)
                return raw, sq

            def norm_rope_two(raw, sq, uid):
                ssq = ps_s.tile([2, TOK], f32, tag="ps", name=f"ssq{uid}")
                nc.tensor.matmul(ssq[:], blk2_sb[:], sq[:], start=True, stop=True)
                rot_ps = pp.tile([P, TOK], f32, tag="pp", name=f"rotp{uid}")
                nc.tensor.matmul(rot_ps[:], rotm_sb[:], raw[:], start=True,
                                 stop=True)
                rot = work.tile([P, TOK], bf16, tag="rot", bufs=WAVE,
                                name=f"rot{uid}")
                nc.vector.tensor_copy(rot[:], rot_ps[:])
                return ssq, rot

            def norm_rope_three(ssq, uid):
                # rsqrt(mean+eps) = exp(-0.5*ln(mean+eps)); Ln/Exp live in
                # the same ACT table set as the attention exps.
                lnt = work.tile([2, TOK], bf16, tag="lnt", name=f"lnt{uid}")
                nc.scalar.activation(
                    lnt[:], ssq[:], AF.Ln, bias=eps_sb[:], scale=1.0 / D
                )
                rs = work.tile([2, TOK], bf16, tag="rs", name=f"rs{uid}")
                nc.scalar.activation(rs[:], lnt[:], AF.Exp, scale=-0.5)
                return rs

            def norm_rope_four(raw, rot, rs, mask_sb, cos_sb, sin_sb, dst,
                               uid):
                bc = pp.tile([P, TOK], f32, tag="pp", name=f"bc{uid}")
                nc.tensor.matmul(bc[:], mask_sb[:], rs[:], start=True, stop=True)
                t1 = work.tile([P, TOK], bf16, tag="t1", name=f"t1{uid}")
                nc.vector.tensor_mul(t1[:], raw[:], cos_sb[:])
                t2 = work.tile([P, TOK], bf16, tag="t2", name=f"t2{uid}")
                nc.vector.tensor_mul(t2[:], rot[:], sin_sb[:])
                u = work.tile([P, TOK], bf16, tag="u", name=f"u{uid}")
                nc.vector.tensor_add(u[:], t1[:], t2[:])
                nc.vector.tensor_mul(dst, u[:], bc[:])

            def norm_wave(pks, mask, cos_sb, sin_sb, dstf, hhs, uid):
                """Norm+rope a wave of head-pair psums, phase-batched so
                each engine's stream stays dense and buffer lifetimes stay
                within the wave."""
                raws, sqs, ssqs, rots, rss = {}, {}, {}, {}, {}
                for hh in hhs:
                    raws[hh], sqs[hh] = norm_rope_one(pks[hh], f"{uid}{hh}")
                for hh in hhs:
                    ssqs[hh], rots[hh] = norm_rope_two(
                        raws[hh], sqs[hh], f"{uid}{hh}"
                    )
                for hh in hhs:
                    rss[hh] = norm_rope_three(ssqs[hh], f"{uid}{hh}")
                for hh in hhs:
                    norm_rope_four(
                        raws[hh], rots[hh], rss[hh], mask, cos_sb, sin_sb,
                        dstf(hh), f"{uid}{hh}",
                    )

            def proj_norm_block(kvsrc_sb, W_t, mask, cos_sb, sin_sb, dstf,
                                uid, wtag="wkq"):
                """Project 16 heads from kvsrc and norm+rope them; dstf(hh)
                gives the destination AP per head-pair."""
                for wave in range(HH // WAVE):
                    pks = {}
                    for g in range(wave * 2, wave * 2 + 2):
                        wk = wpool.tile([P, 2, KO, P], bf16, tag=wtag,
                                        name=f"wk{uid}{g}")
                        nc.sync.dma_start(wk[:], W_t[g])
                        for j in range(2):
                            hh = g * 2 + j
                            pk = pp.tile([P, TOK], f32, tag="pp",
                                         name=f"pk{uid}{hh}")
                            for c in range(KO):
                                nc.tensor.matmul(
                                    pk[:],
                                    wk[:, j, c],
                                    kvsrc_sb[:, c],
                                    start=(c == 0),
                                    stop=(c == KO - 1),
                                )
                            pks[hh] = pk
                    hhs = range(wave * WAVE, (wave + 1) * WAVE)
                    norm_wave(pks, mask, cos_sb, sin_sb, dstf, hhs, uid)

            def rmsnorm_feat(src_f32, dst_bf16, uid):
                """Feature-major RMSNorm: dst = src * rsqrt(mean(src^2))."""
                ssq = ps_s.tile([1, TOK], f32, tag="ps", name=f"yssq{uid}")
                for c in range(KO):
                    ysq = work.tile([P, TOK], bf16, tag="ysq",
                                    name=f"ysq{uid}{c}")
                    nc.vector.tensor_mul(ysq[:], src_f32[:, c], src_f32[:, c])
                    nc.tensor.matmul(
                        ssq[:],
                        ones_c_sb[:],
                        ysq[:],
                        start=(c == 0),
                        stop=(c == KO - 1),
                    )
                lnt = work.tile([1, TOK], f32, tag="lnt", name=f"ylnt{uid}")
                nc.scalar.activation(
                    lnt[:], ssq[:], AF.Ln, bias=eps_sb[:1], scale=1.0 / DIM
                )
                rs = work.tile([1, TOK], bf16, tag="rs", name=f"yrs{uid}")
                nc.scalar.activation(rs[:], lnt[:], AF.Exp, scale=-0.5)
                bc = pp.tile([P, TOK], f32, tag="pp", name=f"ybc{uid}")
                nc.tensor.matmul(
                    bc[:], ones_r128_sb[:], rs[:], start=True, stop=True
                )
                for c in range(KO):
                    nc.vector.tensor_mul(dst_bf16[:, c], src_f32[:, c], bc[:])

            def attention_block(y_fn, kvsrc_fn, Wq_t, Wk_t, Wv_t, Wo_t,
                                mq, mk, cosk, sink, uid, prefetch=None):
                """One attention block. y_fn()/kvsrc_fn() return the q-side /
                kv-side SBUF inputs (y is computed lazily so the kv side can
                be projected and gathered first). Adds Wo output to resid."""
                kvsrc_sb = kvsrc_fn()

                # first v-weight half loads before the k-side DMA below can
                # block the SP DMA queue
                wv0 = wpool.tile([P, KO, TOK], bf16, tag="wv", bufs=1,
                                 name=f"wv{uid}0")
                nc.sync.dma_start(wv0[:], Wv_t[0])

                # --- k projection + norm/rope from my rows (fp8 out)
                k_mine = kvpool.tile([P, HH, TOK], fp8, tag="kmine",
                                     name=f"k_mine{uid}")
                proj_norm_block(
                    kvsrc_sb, Wk_t, mk, cosk, sink,
                    lambda hh: k_mine[:, hh], f"k{uid}",
                )
                kv_in = dram.tile([KWORDS + VWORDS], fp8, tag="kv_in",
                                  name=f"kv_in{uid}")
                # k ships to DRAM while the v projection runs
                nc.sync.dma_start(
                    kv_in[:KWORDS].rearrange(
                        "(p h t) -> p h t", p=P, h=HH, t=TOK
                    ),
                    k_mine[:],
                )

                # --- v projection (token-major, with ones column, fp8)
                v_mine = kvpool.tile(
                    [P, TC, H, VW], fp8, tag="vmine", name=f"v_mine{uid}"
                )
                nc.vector.memset(v_mine[:, :, :, D : D + 1], 1.0)
                for nh in range(2):
                    if nh == 0:
                        wv = wv0
                    else:
                        wv = wpool.tile([P, KO, TOK], bf16, tag="wv", bufs=1,
                                        name=f"wv{uid}{nh}")
                        nc.sync.dma_start(wv[:], Wv_t[nh])
                    for t4 in range(TC):
                        pv = pp.tile([P, TOK], f32, tag="pp",
                                     name=f"pv{uid}{nh}{t4}")
                        for c in range(KO):
                            nc.tensor.matmul(
                                pv[:],
                                kvsrc_sb[:, c, t4 * P : (t4 + 1) * P],
                                wv[:, c],
                                start=(c == 0),
                                stop=(c == KO - 1),
                            )
                        nc.vector.tensor_copy(
                            v_mine[:, t4, nh * 8 : (nh + 1) * 8, 0:D],
                            pv[:].rearrange("p (h d) -> p h d", d=D),
                        )

                # --- allgather k/v across my sample's 4 cores (fp8)
                nc.sync.dma_start(
                    kv_in[KWORDS:].rearrange(
                        "(p a b c) -> p a b c", p=P, a=TC, b=H, c=VW
                    ),
                    v_mine[:],
                )
                kv_out = dram.tile([NR, KWORDS + VWORDS], fp8, tag="kv_out",
                                   name=f"kv_out{uid}")
                nc.gpsimd.collective_compute(
                    "AllGather",
                    mybir.AluOpType.bypass,
                    replica_groups=groups,
                    ins=[kv_in.opt()],
                    outs=[kv_out.opt()],
                )

                # --- wo loads + optional FFN prefetch go on the DMA queue
                # BEFORE the gather-dependent readbacks (SP DMA is FIFO).
                wo_tiles = []
                for g in range(4):
                    wo = wpool.tile([P, 2, KO, P], bf16, tag="wo",
                                    name=f"wo{uid}{g}")
                    nc.sync.dma_start(wo[:], Wo_t[g])
                    wo_tiles.append(wo)
                if prefetch is not None:
                    prefetch()

                # --- y-norm + q projection + norm + rope (overlap gather)
                y_sb = y_fn()
                q_sb = kvpool.tile([P, HH, TOK], fp8, tag="q",
                                   name=f"q_sb{uid}")
                proj_norm_block(
                    y_sb, Wq_t, mq, cosq_sb, sinq_sb,
                    lambda hh: q_sb[:, hh], f"q{uid}",
                )

                # --- gather readback (after all independent DMAs)
                k_full = kvpool.tile(
                    [P, NR, HH, TOK], fp8, tag="kfull", name=f"k_full{uid}"
                )
                v_full = kvpool.tile(
                    [P, NR, TC, H, VW], fp8, tag="vfull", name=f"v_full{uid}"
                )
                for r in range(NR):
                    nc.sync.dma_start(
                        k_full[:, r],
                        kv_out[r, :KWORDS].rearrange(
                            "(p h t) -> p h t", p=P, h=HH, t=TOK
                        ),
                    )
                    nc.sync.dma_start(
                        v_full[:, r],
                        kv_out[r, KWORDS:].rearrange(
                            "(p a b c) -> p a b c", p=P, a=TC, b=H, c=VW
                        ),
                    )

                # --- attention: 2 heads share one exp; denominators ride in
                # row 64 of the px accumulators (ones column of v)
                xT = kvpool.tile([P, HH, TOK], bf16, tag="xT", name=f"xT{uid}")
                for hh in range(HH):
                    px = [
                        ps_x.tile([VW, TOK], f32, tag=f"px{i}",
                                  name=f"px{uid}{hh}{i}")
                        for i in range(2)
                    ]
                    probs = {}
                    # software-pipelined: scores run one chunk ahead of px
                    def scores(kc):
                        r, tcl = kc // TC, kc % TC
                        ps = ps_s.tile([P, 2 * TOK], f32, tag="ps",
                                       name=f"ps{uid}{hh}{kc}")
                        for i in range(2):
                            off = i * D
                            nc.tensor.matmul(
                                ps[:, i * TOK : (i + 1) * TOK],
                                k_full[
                                    off : off + D, r, hh,
                                    tcl * P : (tcl + 1) * P,
                                ],
                                q_sb[off : off + D, hh],
                                start=True,
                                stop=True,
                            )
                        prob = probp.tile([P, 2 * TOK], fp8, tag="prob",
                                          name=f"prob{uid}{hh}{kc}")
                        nc.scalar.activation(
                            prob[:], ps[:], AF.Exp, scale=1.0 / math.sqrt(D)
                        )
                        probs[kc] = prob

                    def pxacc(kc):
                        r, tcl = kc // TC, kc % TC
                        for i in range(2):
                            h = hh * 2 + i
                            nc.tensor.matmul(
                                px[i][:],
                                v_full[:, r, tcl, h],
                                probs[kc][:, i * TOK : (i + 1) * TOK],
                                start=(kc == 0),
                                stop=(kc == H - 1),
                            )

                    scores(0)
                    for kc in range(1, H):
                        scores(kc)
                        pxacc(kc - 1)
                    pxacc(H - 1)

                    # denominators: stay on partition 64, broadcast to all
                    # 128 partitions via a rank-1 matmul (lhsT/rhs both at
                    # base partition 64), invert with 128-lane Ln/Exp.
                    dsb = work.tile([VW, 2, TOK], bf16, tag="dsb", bufs=1,
                                    name=f"dsb{uid}{hh}")
                    for i in range(2):
                        nc.vector.tensor_copy(
                            dsb[D : D + 1, i], px[i][D : D + 1]
                        )
                        nc.vector.tensor_copy(
                            xT[i * D : (i + 1) * D, hh], px[i][0:D]
                        )
                    dbc = ps_s.tile([P, 2 * TOK], f32, tag="ps",
                                    name=f"dbc{uid}{hh}")
                    for i in range(2):
                        nc.tensor.matmul(
                            dbc[:, i * TOK : (i + 1) * TOK],
                            ones64_sb[D : D + 1, :],
                            dsb[D : D + 1, i],
                            start=True,
                            stop=True,
                        )
                    dln = work.tile([P, 2 * TOK], f32, tag="dln", bufs=1,
                                    name=f"dln{uid}{hh}")
                    nc.scalar.activation(dln[:], dbc[:], AF.Ln)
                    rec = work.tile([P, 2 * TOK], bf16, tag="rec",
                                    name=f"rec{uid}{hh}")
                    nc.scalar.activation(rec[:], dln[:], AF.Exp, scale=-1.0)
                    for i in range(2):
                        sl = slice(i * D, (i + 1) * D)
                        nc.vector.tensor_mul(
                            xT[sl, hh], xT[sl, hh],
                            rec[sl, i * TOK : (i + 1) * TOK],
                        )

                # --- Wo projection, accumulate into resid
                for g in range(4):
                    for j in range(2):
                        oc = g * 2 + j
                        po = pp.tile([P, TOK], f32, tag="pp",
                                     name=f"po{uid}{oc}")
                        for c in range(KO):
                            nc.tensor.matmul(
                                po[:],
                                wo_tiles[g][:, j, c],
                                xT[:, c],
                                start=(c == 0),
                                stop=(c == KO - 1),
                            )
                        nc.vector.tensor_add(resid[:, oc], resid[:, oc], po[:])

            # ================= cross-attention =================
            yT = sing.tile([P, KO, TOK], bf16, name="yT")

            def ca_y():
                rmsnorm_feat(resid, yT, "ca")
                return yT

            attention_block(
                ca_y, lambda: srcT_sb, caWq, caWk, caWv, caWo,
                masks_sb["mq_ca"], masks_sb["mk_ca"], coskca_sb, sinkca_sb,
                "ca",
            )

            # ================= self-attention =================
            ffn_pre = {}

            def ffn_prefetch():
                for g in range(2):
                    w1 = wpool.tile([P, 2, KO, P], bf16, tag="w1",
                                    name=f"w1p{g}")
                    nc.sync.dma_start(w1[:], W1i[g])
                    w3 = wpool.tile([P, 2, KO, P], bf16, tag="w3",
                                    name=f"w3p{g}")
                    nc.sync.dma_start(w3[:], W3i[g])
                    ffn_pre[g] = (w1, w3)

            def sa_y():
                rmsnorm_feat(resid, yT, "sa")
                return yT

            sa_y_done = {}

            def sa_y_once():
                if "y" not in sa_y_done:
                    sa_y_done["y"] = sa_y()
                return sa_y_done["y"]

            attention_block(
                sa_y_once, sa_y_once, saWq, saWk, saWv, saWo,
                masks_sb["mq_sa"], masks_sb["mk_sa"], cosq_sb, sinq_sb,
                "sa", prefetch=ffn_prefetch,
            )

            # ================= FFN =================
            rmsnorm_feat(resid, yT, "ffn")
            hT = kvpool.tile([P, HC, TOK], bf16, tag="big", name="hT")
            for gh in range(16):  # half-groups of 2 output chunks
                if gh in ffn_pre:
                    w1, w3 = ffn_pre[gh]
                else:
                    w1 = wpool.tile([P, 2, KO, P], bf16, tag="w1",
                                    name=f"w1{gh}")
                    nc.sync.dma_start(w1[:], W1i[gh])
                    w3 = wpool.tile([P, 2, KO, P], bf16, tag="w3",
                                    name=f"w3{gh}")
                    nc.sync.dma_start(w3[:], W3i[gh])
                for j in range(2):
                    hc = gh * 2 + j
                    p1 = pp.tile([P, TOK], f32, tag="pp", name=f"p1{hc}")
                    for c in range(KO):
                        nc.tensor.matmul(
                            p1[:], w1[:, j, c], yT[:, c],
                            start=(c == 0), stop=(c == KO - 1),
                        )
                    p3 = pp.tile([P, TOK], f32, tag="pp", name=f"p3{hc}")
                    for c in range(KO):
                        nc.tensor.matmul(
                            p3[:], w3[:, j, c], yT[:, c],
                            start=(c == 0), stop=(c == KO - 1),
                        )
                    s1 = work.tile([P, TOK], bf16, tag="s1", name=f"s1{hc}")
                    nc.scalar.activation(s1[:], p1[:], AF.Silu)
                    nc.vector.tensor_mul(hT[:, hc], s1[:], p3[:])
            for oc in range(KO):
                po = ps_s.tile([P, TOK], f32, tag="ps", name=f"po2{oc}")
                for half in range(2):
                    w2 = wpool.tile([P, HC // 2, P], bf16, tag="w2",
                                    name=f"w2{oc}{half}")
                    nc.sync.dma_start(w2[:], W2i[oc, half])
                    for hc in range(HC // 2):
                        hca = half * (HC // 2) + hc
                        nc.tensor.matmul(
                            po[:], w2[:, hc], hT[:, hca],
                            start=(hca == 0), stop=(hca == HC - 1),
                        )
                nc.vector.tensor_add(resid[:, oc], resid[:, oc], po[:])

            nc.sync.dma_start(outT[:], resid[:])

    _split_multiwait(nc)
    return nc


def _prep_inputs(inputs):
    """Full problem inputs -> list of 8 per-core in_maps."""
    tgt = np.asarray(inputs["tgt"], np.float32)
    src = np.asarray(inputs["src"], np.float32)
    tgt_pos = np.asarray(inputs["tgt_pos"], np.int32)
    src_pos = np.asarray(inputs["src_pos"], np.int32)

    pre_ca_w = np.asarray(inputs["pre_ca_w"], np.float32)
    pre_sa_w = np.asarray(inputs["pre_sa_w"], np.float32)
    pre_ffn_w = np.asarray(inputs["pre_ffn_w"], np.float32)

    def fold(Wname, w):
        return np.asarray(inputs[Wname], np.float32) * w[:, None]

    ca_Wq = fold("ca_Wq", pre_ca_w)
    ca_Wkv = np.asarray(inputs["ca_Wkv"], np.float32)
    ca_Wk, ca_Wv = ca_Wkv[:, :DIM], ca_Wkv[:, DIM:]
    ca_Wo = np.asarray(inputs["ca_Wo"], np.float32)
    sa_Wq = fold("sa_Wq", pre_sa_w)
    sa_Wkv = fold("sa_Wkv", pre_sa_w)
    sa_Wk, sa_Wv = sa_Wkv[:, :DIM], sa_Wkv[:, DIM:]
    sa_Wo = np.asarray(inputs["sa_Wo"], np.float32)
    W1 = fold("W1", pre_ffn_w)
    W3 = fold("W3", pre_ffn_w)
    W2 = np.asarray(inputs["W2"], np.float32)

    shared = {
        "caWq": _grouped_lhsT(ca_Wq, 4),
        "caWk": _grouped_lhsT(ca_Wk, 4),
        "caWv": _vrhs_layout(ca_Wv),
        "caWo": _grouped_lhsT(ca_Wo, 4),
        "saWq": _grouped_lhsT(sa_Wq, 4),
        "saWk": _grouped_lhsT(sa_Wk, 4),
        "saWv": _vrhs_layout(sa_Wv),
        "saWo": _grouped_lhsT(sa_Wo, 4),
        "W1": _grouped_lhsT(W1, 16),
        "W3": _grouped_lhsT(W3, 16),
        "W2": _w2_layout(W2).reshape(KO, P, 2, HC // 2, P).transpose(
            0, 2, 1, 3, 4).copy(),
    }

    blk2 = np.zeros((P, 2), BF)
    blk2[:D, 0] = 1
    blk2[D:, 1] = 1
    shared["blk2"] = blk2

    def head_mask(w):  # [2, 128] with per-head norm weight
        m = np.zeros((2, P), np.float32)
        m[0, :D] = w
        m[1, D:] = w
        return m.astype(BF).copy()

    shared["mq_ca"] = head_mask(np.asarray(inputs["ca_qn"], np.float32))
    shared["mk_ca"] = head_mask(np.asarray(inputs["ca_kn"], np.float32))
    shared["mq_sa"] = head_mask(np.asarray(inputs["sa_qn"], np.float32))
    shared["mk_sa"] = head_mask(np.asarray(inputs["sa_kn"], np.float32))

    r64 = np.zeros((D, D), np.float32)
    half = D // 2
    for j in range(half):
        r64[j, j + half] = -1.0  # rot[j] = -x[j+32]
        r64[j + half, j] = 1.0  # rot[j+32] = x[j]
    rt = r64.T  # lhsT (matmul computes lhsT.T @ rhs)
    rotm = np.zeros((P, P), np.float32)
    rotm[:D, :D] = rt
    rotm[D:, D:] = rt
    shared["rotm"] = rotm.astype(BF).copy()

    shared["ones_c"] = np.ones((P, 1), BF)
    shared["ones_r128"] = np.ones((1, P), BF)

    in_maps = []
    for c in range(NCORES):
        s, part = c // NR, c % NR
        rows = slice(part * TOK, (part + 1) * TOK)
        m = dict(shared)
        m["tgtT"] = _featmajor(tgt[s, rows])
        m["srcTb"] = _featmajor(src[s, rows]).astype(BF)
        cq, sq_ = _rope_tables(tgt_pos[s, rows])
        ck, sk = _rope_tables(src_pos[s, rows])
        m["cosq"], m["sinq"] = cq, sq_
        m["coskca"], m["sinkca"] = ck, sk
        in_maps.append(m)
    return in_maps


def _get_nc():
    if "nc" not in _cache:
        _cache["nc"] = _build_bass()
    return _cache["nc"]


def run(inputs, trace=False):
    """Run on 8 cores; returns (full_output, exec_time_ns_or_None)."""
    if trace:
        _install_ntff_hook()
    from concourse.bass_utils import run_bass_kernel_spmd

    in_maps = _prep_inputs(inputs)
    nc = _get_nc()
    res = run_bass_kernel_spmd(
        nc, in_maps, core_ids=list(range(NCORES)), trace=trace
    )
    out = np.empty((B, N, DIM), np.float32)
    for c in range(NCORES):
        s, part = c // NR, c % NR
        arr = np.asarray(res.results[c]["outT"])  # [128, 8, TOK]
        rows = slice(part * TOK, (part + 1) * TOK)
        out[s, rows] = np.transpose(arr, (2, 1, 0)).reshape(TOK, DIM)
    return out, res.exec_time_ns


def kernel(**inputs):
    out, _ = run(inputs, trace=False)
    return out
